# revision 24
# baseline (speedup 1.0000x reference)
"""Coref mention-ranking head on 8 TRN2 NeuronCores (Bass/Tile).

v2 configuration (measured 382767 ns, rel err 1.9224e-2): KF8=4 fp8
DoubleRow tail MMs, ALPHA=1.19 fp8 grid shift via STT X8-prep, psum
in-place adds, relu from psum, lag-2 sgn dot, partition-major DMA
layouts, small-DMAs-first.
"""

from contextlib import ExitStack

import bass_rust
import ml_dtypes
import numpy as np

import concourse.bass as bass
import concourse.bacc as bacc
import concourse.tile as tile
from concourse import mybir
from concourse.bass_utils import run_bass_kernel_spmd

F32 = mybir.dt.float32
BF16 = mybir.dt.bfloat16
F8 = mybir.dt.float8e4
BETA = 16.0
ALPHA = 1.19
KF8 = 4
RELU = mybir.ActivationFunctionType.Relu
ADD = mybir.AluOpType.add
MULT = mybir.AluOpType.mult
DR = mybir.MatmulPerfMode.DoubleRow

N = 2048
H = 1024
A = 50
FEAT = 20
NCORES = 8
NLOC = N // NCORES
W = NLOC + A
KT = H // 128
MT = H // 128
NBLK = A // 2
KB = KT - KF8


def _win(ap2d, base, nseg):
    v = ap2d[:, base:base + nseg + 255].rearrange("p (a b) -> p a b", a=1)
    part = v.ap[0]
    v.ap = bass_rust.VecI64Pair([list(part), [1, nseg], [1, 256]])
    return v


def _bcast(ap2d, lo, nseg):
    return (ap2d[:, lo:lo + 256]
            .rearrange("p (a b) -> p a b", a=1)
            .broadcast_to([128, nseg, 256]))


def _build_nc():
    nc = bacc.Bacc("TRN2", target_bir_lowering=False, debug=False)

    w1db1_d = nc.dram_tensor("w1db1", [FEAT + 1, H], F32, kind="ExternalInput")
    distT1_d = nc.dram_tensor("distT1", [FEAT + 1, A], F32, kind="ExternalInput")
    sgn_d = nc.dram_tensor("sgn", [128, MT], BF16, kind="ExternalInput")
    embT_d = nc.dram_tensor("embT", [128, KT * W], BF16, kind="ExternalInput")
    w1a_d = nc.dram_tensor("w1a", [H, H], BF16, kind="ExternalInput")
    w1b_d = nc.dram_tensor("w1b", [H, H], BF16, kind="ExternalInput")
    w1c_d = nc.dram_tensor("w1c", [128, KB * H], BF16, kind="ExternalInput")
    w1c8_d = nc.dram_tensor("w1c8", [128, KF8 * H], F8, kind="ExternalInput")
    scores_d = nc.dram_tensor("scores", [NBLK, 512], F32, kind="ExternalOutput")

    with tile.TileContext(nc) as tc, ExitStack() as ctx:
        const = ctx.enter_context(tc.tile_pool(name="const", bufs=1))
        wab = ctx.enter_context(tc.tile_pool(name="wab", bufs=3))
        xpool = ctx.enter_context(tc.tile_pool(name="x", bufs=3))
        htpool = ctx.enter_context(tc.tile_pool(name="ht", bufs=12))
        zab_pool = ctx.enter_context(tc.tile_pool(name="zab", bufs=10))

        w1db1_sb = const.tile([FEAT + 1, H], F32)
        nc.sync.dma_start(w1db1_sb[:], w1db1_d[:])
        distT1_sb = const.tile([FEAT + 1, A], F32)
        nc.sync.dma_start(distT1_sb[:], distT1_d[:])
        sgn_sb = const.tile([128, MT], BF16)
        nc.sync.dma_start(sgn_sb[:], sgn_d[:])
        embT = const.tile([128, KT, W], BF16)
        nc.sync.dma_start(embT[:], embT_d.rearrange("p (k w) -> p k w", k=KT))

        zdfb1 = const.tile([128, MT, A], F32)
        zdadj = const.tile([128, MT, 8], F32)
        ZAT = const.tile([128, MT, W], F32)
        ZBT = const.tile([128, MT, W], F32)
        SUB = mybir.AluOpType.subtract
        with tc.tile_pool(name="psum_pre", bufs=8, space="PSUM") as psum_pre:
            for m in range(MT):
                zp = psum_pre.tile([128, A], F32, name=f"zp{m}", tag="zps",
                                   bufs=8)
                nc.tensor.matmul(
                    zp[:], w1db1_sb[:, m * 128:(m + 1) * 128], distT1_sb[:],
                    start=True, stop=True,
                )
                nc.vector.tensor_copy(zdfb1[:, m, :], zp[:])
                nc.vector.tensor_tensor(
                    zdadj[:, m, :], zdfb1[:, m, 0:8],
                    zdfb1[:, m, 8:9].broadcast_to([128, 8]), SUB,
                )

            for wi, (wd, ZT) in enumerate(((w1a_d, ZAT), (w1b_d, ZBT))):
                zps = [psum_pre.tile([128, W], F32, name=f"zps{wi}_{m}",
                                     tag="zps", bufs=8) for m in range(MT)]
                for k in range(KT):
                    wk = wab.tile([128, H], BF16, name=f"wk{wi}_{k}", tag="wk")
                    nc.sync.dma_start(wk[:], wd[k * 128:(k + 1) * 128, :])
                    for m in range(MT):
                        nc.tensor.matmul(
                            zps[m][:],
                            wk[:, m * 128:(m + 1) * 128],
                            embT[:, k, :],
                            start=(k == 0), stop=(k == KT - 1),
                        )
                for m in range(MT):
                    if wi == 0:
                        nc.vector.tensor_copy(ZT[:, m, :], zps[m][:])
                    else:
                        nc.vector.tensor_scalar_add(
                            ZT[:, m, :], zps[m][:], zdfb1[:, m, 8:9],
                        )

        w1c_sb = const.tile([128, KB, H], BF16)
        nc.sync.dma_start(w1c_sb[:], w1c_d.rearrange("p (k h) -> p k h", k=KB))
        w1c8_sb = const.tile([128, KF8, H], F8)
        nc.sync.dma_start(w1c8_sb[:], w1c8_d.rearrange("p (k h) -> p k h", k=KF8))

        psum_main = ctx.enter_context(
            tc.tile_pool(name="psum_main", bufs=6, space="PSUM"))
        groups = [(4 * p + 1, 4) for p in range(NBLK // 2)] + [(A - 1, 2)]
        for g, (sBot, nseg) in enumerate(groups):
            sTop = sBot + nseg - 1
            base = A - sTop
            nwide = nseg * 256
            X8 = xpool.tile([128, KF8, nwide], F8, name=f"X8{g}", tag="X8",
                            bufs=2)
            X = xpool.tile([128, KB, nwide], BF16, name=f"X{g}",
                           tag="X", bufs=2)
            for k in range(KT):
                if k < KF8:
                    nc.vector.scalar_tensor_tensor(
                        X8[:, k, :].rearrange("p (j i) -> p j i", j=nseg),
                        _bcast(embT[:, k, :], A, nseg),
                        1.0 / ALPHA,
                        _win(embT[:, k, :], base, nseg),
                        MULT, MULT,
                    )
                else:
                    nc.vector.tensor_tensor(
                        X[:, k - KF8, :].rearrange("p (j i) -> p j i", j=nseg),
                        _bcast(embT[:, k, :], A, nseg),
                        _win(embT[:, k, :], base, nseg),
                        MULT,
                    )
            sps = [psum_main.tile([1, 512], F32, name=f"sps{g}_{h}",
                                  tag="sps", bufs=2) for h in range(nseg // 2)]
            pend = []
            for m in range(MT):
                ps = psum_main.tile([128, nwide], F32, name=f"ps{g}_{m}",
                                    tag="ps", bufs=3)
                for half in range(nwide // 512):
                    hs = slice(half * 512, (half + 1) * 512)
                    for k in range(KB):
                        nc.tensor.matmul(
                            ps[:, hs],
                            w1c_sb[:, k, m * 128:(m + 1) * 128],
                            X[:, k, hs],
                            start=(k == 0), stop=False,
                        )
                    for t in range(KF8 // 2):
                        nc.tensor.matmul(
                            ps[:, hs],
                            w1c8_sb[:, 2 * t:2 * t + 2, m * 128:(m + 1) * 128],
                            X8[:, 2 * t:2 * t + 2, hs],
                            start=False, stop=(t == KF8 // 2 - 1),
                            perf_mode=DR,
                        )
                psn = ps[:].rearrange("p (j i) -> p j i", j=nseg)
                if sBot >= 9:
                    zab = zab_pool.tile([128, nseg, 256], F32,
                                        name=f"zab{g}_{m}", tag="zab")
                    nc.gpsimd.tensor_tensor(
                        zab[:],
                        _win(ZBT[:, m, :], base, nseg),
                        _bcast(ZAT[:, m, :], A, nseg),
                        ADD,
                    )
                    nc.vector.tensor_add(psn, psn, zab[:])
                else:
                    for j in range(nseg):
                        s = sTop - j
                        nc.vector.scalar_tensor_tensor(
                            ps[:, j * 256:(j + 1) * 256],
                            ps[:, j * 256:(j + 1) * 256],
                            zdadj[:, m, s - 1:s],
                            ZBT[:, m, base + j:base + j + 256],
                            ADD, ADD,
                        )
                    nc.vector.tensor_add(psn, psn, _bcast(ZAT[:, m, :], A, nseg))
                ht = htpool.tile([128, nseg, 256], BF16, name=f"ht{g}_{m}",
                                 tag="ht")
                nc.scalar.activation(ht[:], psn, RELU)
                pend.append((m, ht))
                if len(pend) > 2:
                    pm, pht = pend.pop(0)
                    for h in range(nseg // 2):
                        nc.tensor.matmul(
                            sps[h][:], sgn_sb[:, pm:pm + 1],
                            pht[:, 2 * h:2 * h + 2, :],
                            start=(pm == 0), stop=False,
                        )
            for pm, pht in pend:
                for h in range(nseg // 2):
                    nc.tensor.matmul(
                        sps[h][:], sgn_sb[:, pm:pm + 1],
                        pht[:, 2 * h:2 * h + 2, :],
                        start=(pm == 0), stop=(pm == MT - 1),
                    )
            for h in range(nseg // 2):
                blk = (sTop - 2 * h - 1) // 2
                srow = htpool.tile([1, 512], F32, name=f"srow{g}_{h}",
                                   tag="srow", bufs=3)
                nc.scalar.copy(srow[:], sps[h][:])
                nc.sync.dma_start(scores_d[blk:blk + 1, :], srow[:])

    nc.compile()
    if not nc.is_finalized():
        nc.finalize()
    return nc


def _host_prep(mention_embeddings, W1, b1, W2, dist_emb):
    emb = np.asarray(mention_embeddings, dtype=np.float32)
    W1 = np.asarray(W1, dtype=np.float32)
    b1 = np.asarray(b1, dtype=np.float32)
    W2 = np.asarray(W2, dtype=np.float32)
    dist_emb = np.asarray(dist_emb, dtype=np.float32)

    W1s = W1 * BETA
    b1s = b1 * BETA
    w2v = W2 / BETA

    w1a = np.ascontiguousarray(W1s[0:H].astype(ml_dtypes.bfloat16))
    w1b = np.ascontiguousarray(W1s[H:2 * H].astype(ml_dtypes.bfloat16))
    w1c_kt = W1s[2 * H:3 * H].reshape(KT, 128, H)
    w1c = np.ascontiguousarray(
        w1c_kt[KF8:].transpose(1, 0, 2).reshape(128, KB * H)
        .astype(ml_dtypes.bfloat16))
    w1c8 = np.ascontiguousarray(
        np.clip(w1c_kt[:KF8] * ALPHA, -240, 240)
        .transpose(1, 0, 2).reshape(128, KF8 * H)
        .astype(ml_dtypes.float8_e4m3))
    w1db1 = np.ascontiguousarray(
        np.concatenate([W1s[3 * H:], b1s[None, :]], axis=0))
    svals = np.arange(1, A + 1)
    distT1 = np.ascontiguousarray(np.concatenate(
        [dist_emb[np.minimum(svals, 9)].T, np.ones((1, A), np.float32)], axis=0))
    sgn_in = np.ascontiguousarray(w2v.reshape(MT, 128).T.astype(ml_dtypes.bfloat16))

    embTfull = np.zeros((H, N + A), dtype=ml_dtypes.bfloat16)
    embTfull[:, A:] = emb.T.astype(ml_dtypes.bfloat16)

    in_maps = []
    for r in range(NCORES):
        n0 = r * NLOC
        embw = embTfull[:, n0:n0 + W].reshape(KT, 128, W)
        in_maps.append({
            "embT": np.ascontiguousarray(
                embw.transpose(1, 0, 2).reshape(128, KT * W)),
            "w1a": w1a, "w1b": w1b, "w1c": w1c, "w1c8": w1c8,
            "w1db1": w1db1, "distT1": distT1, "sgn": sgn_in,
        })
    return in_maps


def _assemble(grids, b2):
    b2v = np.float32(np.asarray(b2).reshape(-1)[0])
    grid = np.concatenate(
        [g.reshape(NBLK, 2, NLOC)[:, ::-1].reshape(A, NLOC) for g in grids],
        axis=1)
    out = np.zeros((N, A + 1), dtype=np.float32)
    big = grid[::-1].T + b2v
    out[A:, 1:] = big[A:]
    for i in range(1, A):
        ss = np.arange(1, i + 1)
        out[i, 1 + (i - ss)] = grid[ss - 1, i] + b2v
    return out


def kernel(mention_embeddings, mention_indices, max_antecedents, W1, b1, W2,
           b2, dist_emb):
    assert int(max_antecedents) == A
    in_maps = _host_prep(mention_embeddings, W1, b1, W2, dist_emb)
    nc = _build_nc()
    res = run_bass_kernel_spmd(nc, in_maps, list(range(NCORES)))
    grids = [res.results[r]["scores"] for r in range(NCORES)]
    return _assemble(grids, b2)


# revision 26
# speedup vs baseline: 1.1902x; 1.1902x over previous
"""Coref mention-ranking head on 8 TRN2 NeuronCores (Bass/Tile).

v2 configuration (measured 382767 ns, rel err 1.9224e-2): KF8=4 fp8
DoubleRow tail MMs, ALPHA=1.19 fp8 grid shift via STT X8-prep, psum
in-place adds, relu from psum, lag-2 sgn dot, partition-major DMA
layouts, small-DMAs-first.
"""

from contextlib import ExitStack

import bass_rust
import ml_dtypes
import numpy as np

import concourse.bass as bass
import concourse.bacc as bacc
import concourse.tile as tile
from concourse import mybir
from concourse.bass_utils import run_bass_kernel_spmd

F32 = mybir.dt.float32
BF16 = mybir.dt.bfloat16
F8 = mybir.dt.float8e4
BETA = 16.0
ALPHA = 1.19
KF8 = 4
RELU = mybir.ActivationFunctionType.Relu
ADD = mybir.AluOpType.add
MULT = mybir.AluOpType.mult
DR = mybir.MatmulPerfMode.DoubleRow

N = 2048
H = 1024
A = 50
FEAT = 20
NCORES = 8
NLOC = N // NCORES
W = NLOC + A
KT = H // 128
MT = H // 128
NBLK = A // 2
KB = KT - KF8


def _win(ap2d, base, nseg):
    v = ap2d[:, base:base + nseg + 255].rearrange("p (a b) -> p a b", a=1)
    part = v.ap[0]
    v.ap = bass_rust.VecI64Pair([list(part), [1, nseg], [1, 256]])
    return v


def _bcast(ap2d, lo, nseg):
    return (ap2d[:, lo:lo + 256]
            .rearrange("p (a b) -> p a b", a=1)
            .broadcast_to([128, nseg, 256]))


def _build_nc():
    nc = bacc.Bacc("TRN2", target_bir_lowering=False, debug=False)

    w1db1_d = nc.dram_tensor("w1db1", [FEAT + 1, H], F32, kind="ExternalInput")
    distT1_d = nc.dram_tensor("distT1", [FEAT + 1, A], F32, kind="ExternalInput")
    sgn_d = nc.dram_tensor("sgn", [128, MT], BF16, kind="ExternalInput")
    embT_d = nc.dram_tensor("embT", [128, KT * W], BF16, kind="ExternalInput")
    w1a_d = nc.dram_tensor("w1a", [H, H], BF16, kind="ExternalInput")
    w1b_d = nc.dram_tensor("w1b", [H, H], BF16, kind="ExternalInput")
    w1c_d = nc.dram_tensor("w1c", [128, KB * H], BF16, kind="ExternalInput")
    w1c8_d = nc.dram_tensor("w1c8", [128, KF8 * H], F8, kind="ExternalInput")
    scores_d = nc.dram_tensor("scores", [NBLK, 512], F32, kind="ExternalOutput")

    with tile.TileContext(nc) as tc, ExitStack() as ctx:
        const = ctx.enter_context(tc.tile_pool(name="const", bufs=1))
        wab = ctx.enter_context(tc.tile_pool(name="wab", bufs=3))
        xpool = ctx.enter_context(tc.tile_pool(name="x", bufs=3))
        htpool = ctx.enter_context(tc.tile_pool(name="ht", bufs=12))
        zab_pool = ctx.enter_context(tc.tile_pool(name="zab", bufs=10))

        w1db1_sb = const.tile([FEAT + 1, H], F32)
        nc.sync.dma_start(w1db1_sb[:], w1db1_d[:])
        distT1_sb = const.tile([FEAT + 1, A], F32)
        nc.sync.dma_start(distT1_sb[:], distT1_d[:])
        sgn_sb = const.tile([128, MT], BF16)
        nc.sync.dma_start(sgn_sb[:], sgn_d[:])
        embT = const.tile([128, KT, W], BF16)
        nc.sync.dma_start(embT[:], embT_d.rearrange("p (k w) -> p k w", k=KT))

        zdfb1 = const.tile([128, MT, A], F32)
        zdadj = const.tile([128, MT, 8], F32)
        ZAT = const.tile([128, MT, W], F32)
        ZBT = const.tile([128, MT, W], F32)
        SUB = mybir.AluOpType.subtract
        with tc.tile_pool(name="psum_pre", bufs=8, space="PSUM") as psum_pre:
            for m in range(MT):
                zp = psum_pre.tile([128, A], F32, name=f"zp{m}", tag="zps",
                                   bufs=8)
                nc.tensor.matmul(
                    zp[:], w1db1_sb[:, m * 128:(m + 1) * 128], distT1_sb[:],
                    start=True, stop=True,
                )
                nc.vector.tensor_copy(zdfb1[:, m, :], zp[:])
                nc.vector.tensor_tensor(
                    zdadj[:, m, :], zdfb1[:, m, 0:8],
                    zdfb1[:, m, 8:9].broadcast_to([128, 8]), SUB,
                )

            for wi, (wd, ZT) in enumerate(((w1a_d, ZAT), (w1b_d, ZBT))):
                zps = [psum_pre.tile([128, W], F32, name=f"zps{wi}_{m}",
                                     tag="zps", bufs=8) for m in range(MT)]
                for k in range(KT):
                    wk = wab.tile([128, H], BF16, name=f"wk{wi}_{k}", tag="wk")
                    # scalar-engine DMA queue: streams in parallel with the
                    # sync-queue loads (embT/w1c) instead of behind them
                    nc.scalar.dma_start(wk[:], wd[k * 128:(k + 1) * 128, :])
                    for m in range(MT):
                        nc.tensor.matmul(
                            zps[m][:],
                            wk[:, m * 128:(m + 1) * 128],
                            embT[:, k, :],
                            start=(k == 0), stop=(k == KT - 1),
                        )
                for m in range(MT):
                    if wi == 0:
                        nc.vector.tensor_copy(ZT[:, m, :], zps[m][:])
                    else:
                        nc.vector.tensor_scalar_add(
                            ZT[:, m, :], zps[m][:], zdfb1[:, m, 8:9],
                        )

        w1c_sb = const.tile([128, KB, H], BF16)
        nc.sync.dma_start(w1c_sb[:], w1c_d.rearrange("p (k h) -> p k h", k=KB))
        w1c8_sb = const.tile([128, KF8, H], F8)
        nc.sync.dma_start(w1c8_sb[:], w1c8_d.rearrange("p (k h) -> p k h", k=KF8))

        psum_main = ctx.enter_context(
            tc.tile_pool(name="psum_main", bufs=6, space="PSUM"))
        groups = [(4 * p + 1, 4) for p in range(NBLK // 2)] + [(A - 1, 2)]
        for g, (sBot, nseg) in enumerate(groups):
            sTop = sBot + nseg - 1
            base = A - sTop
            nwide = nseg * 256
            X8 = xpool.tile([128, KF8, nwide], F8, name=f"X8{g}", tag="X8",
                            bufs=2)
            X = xpool.tile([128, KB, nwide], BF16, name=f"X{g}",
                           tag="X", bufs=2)
            for k in range(KT):
                if k < KF8:
                    nc.vector.scalar_tensor_tensor(
                        X8[:, k, :].rearrange("p (j i) -> p j i", j=nseg),
                        _bcast(embT[:, k, :], A, nseg),
                        1.0 / ALPHA,
                        _win(embT[:, k, :], base, nseg),
                        MULT, MULT,
                    )
                else:
                    nc.vector.tensor_tensor(
                        X[:, k - KF8, :].rearrange("p (j i) -> p j i", j=nseg),
                        _bcast(embT[:, k, :], A, nseg),
                        _win(embT[:, k, :], base, nseg),
                        MULT,
                    )
            sps = [psum_main.tile([1, 512], F32, name=f"sps{g}_{h}",
                                  tag="sps", bufs=2) for h in range(nseg // 2)]
            pend = []
            for m in range(MT):
                ps = psum_main.tile([128, nwide], F32, name=f"ps{g}_{m}",
                                    tag="ps", bufs=3)
                for half in range(nwide // 512):
                    hs = slice(half * 512, (half + 1) * 512)
                    for k in range(KB):
                        nc.tensor.matmul(
                            ps[:, hs],
                            w1c_sb[:, k, m * 128:(m + 1) * 128],
                            X[:, k, hs],
                            start=(k == 0), stop=False,
                        )
                    for t in range(KF8 // 2):
                        nc.tensor.matmul(
                            ps[:, hs],
                            w1c8_sb[:, 2 * t:2 * t + 2, m * 128:(m + 1) * 128],
                            X8[:, 2 * t:2 * t + 2, hs],
                            start=False, stop=(t == KF8 // 2 - 1),
                            perf_mode=DR,
                        )
                psn = ps[:].rearrange("p (j i) -> p j i", j=nseg)
                if sBot >= 9:
                    zab = zab_pool.tile([128, nseg, 256], F32,
                                        name=f"zab{g}_{m}", tag="zab")
                    nc.gpsimd.tensor_tensor(
                        zab[:],
                        _win(ZBT[:, m, :], base, nseg),
                        _bcast(ZAT[:, m, :], A, nseg),
                        ADD,
                    )
                    nc.vector.tensor_add(psn, psn, zab[:])
                else:
                    for j in range(nseg):
                        s = sTop - j
                        nc.vector.scalar_tensor_tensor(
                            ps[:, j * 256:(j + 1) * 256],
                            ps[:, j * 256:(j + 1) * 256],
                            zdadj[:, m, s - 1:s],
                            ZBT[:, m, base + j:base + j + 256],
                            ADD, ADD,
                        )
                    nc.vector.tensor_add(psn, psn, _bcast(ZAT[:, m, :], A, nseg))
                ht = htpool.tile([128, nseg, 256], BF16, name=f"ht{g}_{m}",
                                 tag="ht")
                nc.scalar.activation(ht[:], psn, RELU)
                pend.append((m, ht))
                if len(pend) > 3:
                    pm, pht = pend.pop(0)
                    for h in range(nseg // 2):
                        nc.tensor.matmul(
                            sps[h][:], sgn_sb[:, pm:pm + 1],
                            pht[:, 2 * h:2 * h + 2, :],
                            start=(pm == 0), stop=False,
                        )
            for pm, pht in pend:
                for h in range(nseg // 2):
                    nc.tensor.matmul(
                        sps[h][:], sgn_sb[:, pm:pm + 1],
                        pht[:, 2 * h:2 * h + 2, :],
                        start=(pm == 0), stop=(pm == MT - 1),
                    )
            for h in range(nseg // 2):
                blk = (sTop - 2 * h - 1) // 2
                srow = htpool.tile([1, 512], F32, name=f"srow{g}_{h}",
                                   tag="srow", bufs=3)
                nc.scalar.copy(srow[:], sps[h][:])
                nc.sync.dma_start(scores_d[blk:blk + 1, :], srow[:])

    nc.compile()
    if not nc.is_finalized():
        nc.finalize()
    return nc


def _host_prep(mention_embeddings, W1, b1, W2, dist_emb):
    emb = np.asarray(mention_embeddings, dtype=np.float32)
    W1 = np.asarray(W1, dtype=np.float32)
    b1 = np.asarray(b1, dtype=np.float32)
    W2 = np.asarray(W2, dtype=np.float32)
    dist_emb = np.asarray(dist_emb, dtype=np.float32)

    W1s = W1 * BETA
    b1s = b1 * BETA
    w2v = W2 / BETA

    w1a = np.ascontiguousarray(W1s[0:H].astype(ml_dtypes.bfloat16))
    w1b = np.ascontiguousarray(W1s[H:2 * H].astype(ml_dtypes.bfloat16))
    w1c_kt = W1s[2 * H:3 * H].reshape(KT, 128, H)
    w1c = np.ascontiguousarray(
        w1c_kt[KF8:].transpose(1, 0, 2).reshape(128, KB * H)
        .astype(ml_dtypes.bfloat16))
    w1c8 = np.ascontiguousarray(
        np.clip(w1c_kt[:KF8] * ALPHA, -240, 240)
        .transpose(1, 0, 2).reshape(128, KF8 * H)
        .astype(ml_dtypes.float8_e4m3))
    w1db1 = np.ascontiguousarray(
        np.concatenate([W1s[3 * H:], b1s[None, :]], axis=0))
    svals = np.arange(1, A + 1)
    distT1 = np.ascontiguousarray(np.concatenate(
        [dist_emb[np.minimum(svals, 9)].T, np.ones((1, A), np.float32)], axis=0))
    sgn_in = np.ascontiguousarray(w2v.reshape(MT, 128).T.astype(ml_dtypes.bfloat16))

    embTfull = np.zeros((H, N + A), dtype=ml_dtypes.bfloat16)
    embTfull[:, A:] = emb.T.astype(ml_dtypes.bfloat16)

    in_maps = []
    for r in range(NCORES):
        n0 = r * NLOC
        embw = embTfull[:, n0:n0 + W].reshape(KT, 128, W)
        in_maps.append({
            "embT": np.ascontiguousarray(
                embw.transpose(1, 0, 2).reshape(128, KT * W)),
            "w1a": w1a, "w1b": w1b, "w1c": w1c, "w1c8": w1c8,
            "w1db1": w1db1, "distT1": distT1, "sgn": sgn_in,
        })
    return in_maps


def _assemble(grids, b2):
    b2v = np.float32(np.asarray(b2).reshape(-1)[0])
    grid = np.concatenate(
        [g.reshape(NBLK, 2, NLOC)[:, ::-1].reshape(A, NLOC) for g in grids],
        axis=1)
    out = np.zeros((N, A + 1), dtype=np.float32)
    big = grid[::-1].T + b2v
    out[A:, 1:] = big[A:]
    for i in range(1, A):
        ss = np.arange(1, i + 1)
        out[i, 1 + (i - ss)] = grid[ss - 1, i] + b2v
    return out


def kernel(mention_embeddings, mention_indices, max_antecedents, W1, b1, W2,
           b2, dist_emb):
    assert int(max_antecedents) == A
    in_maps = _host_prep(mention_embeddings, W1, b1, W2, dist_emb)
    nc = _build_nc()
    res = run_bass_kernel_spmd(nc, in_maps, list(range(NCORES)))
    grids = [res.results[r]["scores"] for r in range(NCORES)]
    return _assemble(grids, b2)
